# revision 20
# baseline (speedup 1.0000x reference)
"""Multi-head attention (B=2, N=2048, D=1024, H=16) on 8 TRN2 NeuronCores.

Sharding: tensor-parallel over heads. Core c owns heads 2c, 2c+1 (a 128-wide
slice of the concat head dim). Host sums the 8 partial outputs and adds bo.

Structure (v4 — ACT-paced co-schedule):
  - K/Q/V all projected in transposed layout [dh, rows] via 512-col-stream
    matmuls (x moving, weights stationary).  V^T is then turned into PV
    layout [keys, dh] with PE identity-transposes ([128,128] bf16, ~250ns
    each) + two DVE copies per 128-key block — this avoids both the
    ldweights-bound natural V projection and the 1.2us-per-issue XBAR
    transpose DMAs that serialize the sync queue.
  - bk is dropped (softmax-invariant); bv is folded into bo on the host
    (bo' = bo + Wo @ bv); bq is applied in the Q psum->sbuf copy.
  - scores S^T[k,q] for the two heads run CONCURRENTLY on disjoint PE
    row-groups (64-deep contraction, auto tile_position).  exp on ScalarE
    (scale=1/8 folded, no max-subtract), denominator via ones columns in the
    V tile (col 64 for h0, col 160 for h1 -> sumexp at psum row 64 for both).
  - softmax normalize: copy sumexp psum row -> sbuf partition 0, fast
    reciprocal, gpsimd partition_broadcast, one DVE multiply per head
    writing xT (h1 is a cross-partition write, psum parts 0:64 -> 64:128).
  - The exp stream (ACT, ~1.1us per key-tile, 142us total) and the PE
    matmul stream (~145us) are nearly equal; attention key-tiles and
    projection / out-projection units are interleaved at ~0.4us granularity
    so neither engine starves (PE idle triggers HAM half-clock throttling).
    qt0 of each batch runs a fixed JIT schedule chasing the x DMAs (1MB
    half-tile loads, no per-chunk splits: each dma_start costs ~1.1us of
    queue issue time); out-projections are deferred to pad the tail.
"""

import sys

sys.path.insert(0, "/opt/trn_rl_repo")

from collections import deque
from contextlib import ExitStack

import ml_dtypes
import numpy as np

import concourse.bass as bass
import concourse.mybir as mybir
import concourse.tile as tile
from concourse import bacc
from concourse.bass_utils import run_bass_kernel_spmd

B, N, D, H, DH = 2, 2048, 1024, 16, 64
R = B * N  # 4096
NC = 8
HPC = H // NC  # 2 heads per core
DHC = HPC * DH  # 128 head dims per core
QT = 512  # query tile (psum bank)
KT = 128  # key tile (psum partitions)
NQT = N // QT  # 4
NKT = N // KT  # 16
KC = D // 128  # 8 contraction chunks
XW = 2048  # rows per x tile (full batch)

f32 = mybir.dt.float32
bf16 = mybir.dt.bfloat16
EXP = mybir.ActivationFunctionType.Exp

_cache = {}


def _fold(ap):
    # [D, X] dram -> [128, KC, X] partition-folded view
    return ap.rearrange("(a p) m -> p a m", p=128)


def _foldw(w):
    # [D, DHC] host weight -> [128, KC, DHC] partition-folded, contiguous
    return np.ascontiguousarray(w.reshape(KC, 128, DHC).transpose(1, 0, 2))


def build():
    if "nc" in _cache:
        return _cache["nc"]
    nc = bacc.Bacc("TRN2", target_bir_lowering=False, debug=False, num_devices=NC)
    xq = nc.dram_tensor("xqT", [D, R], bf16, kind="ExternalInput").ap()
    xk = nc.dram_tensor("xkT", [D, R], bf16, kind="ExternalInput").ap()
    xv = nc.dram_tensor("xvT", [D, R], bf16, kind="ExternalInput").ap()
    wq = nc.dram_tensor("wqT", [128, KC, DHC], bf16, kind="ExternalInput").ap()
    wk = nc.dram_tensor("wkT", [128, KC, DHC], bf16, kind="ExternalInput").ap()
    wv = nc.dram_tensor("wvT", [128, KC, DHC], bf16, kind="ExternalInput").ap()
    wo = nc.dram_tensor("woT", [DHC, D], bf16, kind="ExternalInput").ap()
    bq = nc.dram_tensor("bq", [DHC, 1], f32, kind="ExternalInput").ap()
    ident = nc.dram_tensor("ident", [128, 128], bf16, kind="ExternalInput").ap()
    outT = nc.dram_tensor("outT", [D, R], bf16, kind="ExternalOutput").ap()

    with tile.TileContext(nc) as tc, ExitStack() as ctx:
        const = ctx.enter_context(tc.tile_pool(name="const", bufs=1))
        xpool = ctx.enter_context(tc.tile_pool(name="x", bufs=3))
        big = ctx.enter_context(tc.tile_pool(name="big", bufs=1))
        vtp = ctx.enter_context(tc.tile_pool(name="vt", bufs=2))
        ppool = ctx.enter_context(tc.tile_pool(name="p", bufs=4))
        obp = ctx.enter_context(tc.tile_pool(name="o", bufs=3))
        npool = ctx.enter_context(tc.tile_pool(name="norm", bufs=2))
        ps_sg = ctx.enter_context(tc.tile_pool(name="psS", bufs=2, space="PSUM"))
        ps_pv = ctx.enter_context(tc.tile_pool(name="psPV", bufs=2, space="PSUM"))
        # proj accumulators, out-proj tiles and V-transposes share this pool
        ps_pj = ctx.enter_context(tc.tile_pool(name="psPJ", bufs=2, space="PSUM"))

        # ---- constants ----
        wk_sb = const.tile([128, KC, DHC], bf16, tag="wk")
        nc.sync.dma_start(wk_sb[:], wk)
        wq_sb = const.tile([128, KC, DHC], bf16, tag="wq")
        nc.sync.dma_start(wq_sb[:], wq)
        wv_sb = const.tile([128, KC, DHC], bf16, tag="wv")
        nc.sync.dma_start(wv_sb[:], wv)
        bq_sb = const.tile([DHC, 1], f32, tag="bq")
        nc.sync.dma_start(bq_sb[:], bq)
        id_sb = const.tile([128, 128], bf16, tag="id")
        nc.sync.dma_start(id_sb[:], ident)

        # ---- persistent activations ----
        qTs, kTs, vss, xTs = [], [], [], []
        for b in range(B):
            qTs.append(big.tile([128, N], bf16, tag=f"qT{b}", name=f"qT{b}"))
            kTs.append(big.tile([128, N], bf16, tag=f"kT{b}", name=f"kT{b}"))
            # per key block: cols 0:64 h0 dims, ones @64, 96:160 h1 dims, ones @160
            v = big.tile([128, NKT, 192], bf16, tag=f"v{b}", name=f"v{b}")
            nc.vector.memset(v[:], 1.0)
            vss.append(v)
            xTs.append(big.tile([128, N], bf16, tag=f"xT{b}", name=f"xT{b}"))

        # ---- x tile management (1MB half-loads / 2MB full loads) ----
        xts = {}

        def xtile(nm, b):
            key = (nm, b)
            if key not in xts:
                xts[key] = xpool.tile([128, KC, XW], bf16, tag="x", name=f"x_{nm}{b}")
            return xts[key]

        # all x loads ride the sync HWDGE in first-need order; aggregate
        # HBM read bandwidth (~350GB/s) is shared across queues, so
        # splitting across queues only fair-shares it away from the
        # critical first tiles (measured: strictly worse)
        def xdma(nm, xdram, b, kind="full"):
            def u():
                xt = xtile(nm, b)
                # b1's x_v overlaps on the activation HWDGE: issued mid-run
                # with its buffer already free (no WAR wait to stall exps),
                # it runs parallel to the sync queue's xk1/xq1 so batch-1
                # data lands before the b0->b1 boundary
                eng = nc.scalar if (nm == "v" and b == 1) else nc.sync
                if kind == "h0":
                    eng.dma_start(
                        xt[:, :, 0:1024], _fold(xdram[:, b * N : b * N + 1024])
                    )
                elif kind == "h1":
                    eng.dma_start(
                        xt[:, :, 1024:2048],
                        _fold(xdram[:, b * N + 1024 : b * N + 2048]),
                    )
                elif kind == "a3":
                    eng.dma_start(
                        xt[:, :, 0:1536], _fold(xdram[:, b * N : b * N + 1536])
                    )
                elif kind == "b1":
                    eng.dma_start(
                        xt[:, :, 1536:2048],
                        _fold(xdram[:, b * N + 1536 : b * N + 2048]),
                    )
                else:
                    eng.dma_start(xt[:], _fold(xdram[:, b * N : b * N + XW]))

            return u

        # ---- projection quarter-units (2 contraction chunks each) ----
        def proj4(nm, b, t, w_sb, bias, dst_fn):
            st = {}

            def mk(i):
                def u():
                    x = xtile(nm, b)
                    col0 = t * QT
                    if i == 0:
                        st["ps"] = ps_pj.tile([128, QT], f32, tag="pj", name="pspj")
                    ps = st["ps"]
                    for kc in (2 * i, 2 * i + 1):
                        nc.tensor.matmul(
                            ps[:],
                            w_sb[:, kc, :],
                            x[:, kc, col0 : col0 + QT],
                            start=(kc == 0),
                            stop=(kc == KC - 1),
                        )
                    if i == 3:
                        dst = dst_fn()
                        if bias is None:
                            nc.vector.tensor_copy(dst, ps[:])
                        else:
                            nc.vector.tensor_scalar_add(dst, ps[:], bias[:])

                return u

            return [mk(i) for i in range(4)]

        vts = {}

        def vtile(b):
            if b not in vts:
                vts[b] = vtp.tile([128, N], bf16, tag="vt", name=f"vT{b}")
            return vts[b]

        def vt4(b, t):
            return proj4(
                "v", b, t, wv_sb, None, lambda: vtile(b)[:, t * QT : (t + 1) * QT]
            )

        def tp1(b, blk):
            # PE transpose V^T block [128 dhc, 128 keys] -> [keys, dhc] psum,
            # then split the two heads' halves into the vss PV layout
            def u():
                tp = ps_pj.tile([128, 128], bf16, tag="pj", name="pstp")
                nc.tensor.transpose(
                    tp[:], vtile(b)[:, blk * KT : (blk + 1) * KT], id_sb[:]
                )
                nc.vector.tensor_copy(vss[b][:, blk, 0:64], tp[:, 0:64])
                nc.vector.tensor_copy(vss[b][:, blk, 96:160], tp[:, 64:128])

            return u

        def out_unit(b, qt, ot, scalar_copy=False):
            def u():
                ps = ps_pj.tile([128, QT], f32, tag="pj", name="pso")
                nc.tensor.matmul(
                    ps[:],
                    wo_sb[:, ot * 128 : (ot + 1) * 128],
                    xTs[b][:, qt * QT : (qt + 1) * QT],
                    start=True,
                    stop=True,
                )
                ob = obp.tile([128, QT], bf16, tag="ob", name="ob")
                if scalar_copy:
                    nc.scalar.copy(ob[:], ps[:])
                else:
                    nc.vector.tensor_copy(ob[:], ps[:])
                nc.gpsimd.dma_start(
                    outT[
                        ot * 128 : (ot + 1) * 128,
                        b * N + qt * QT : b * N + (qt + 1) * QT,
                    ],
                    ob[:],
                )

            return u

        QuCrit = deque()
        QuSoft = deque()

        def pump(n):
            for _ in range(n):
                if QuCrit:
                    QuCrit.popleft()()
                elif QuSoft:
                    QuSoft.popleft()()

        def drain_crit():
            while QuCrit:
                QuCrit.popleft()()

        # ---- attention for one query tile ----
        def attention_qt(b, qt, sched=None, head=None):
            qs = slice(qt * QT, (qt + 1) * QT)
            pv = [
                ps_pv.tile([65, QT], f32, tag="pv", name=f"pv{h}") for h in range(HPC)
            ]
            pts = {}

            def scores_exp(kt):
                ks = slice(kt * KT, (kt + 1) * KT)
                sg = ps_sg.tile([128, 2 * QT], f32, tag="sg", name="sg")
                for h in range(HPC):
                    hp = slice(64 * h, 64 * h + 64)
                    nc.tensor.matmul(
                        sg[:, h * QT : (h + 1) * QT],
                        kTs[b][hp, ks],
                        qTs[b][hp, qs],
                        start=True,
                        stop=True,
                    )
                pt = ppool.tile([128, 2 * QT], bf16, tag="pt", name="pt")
                nc.scalar.activation(pt[:], sg[:], EXP, scale=0.125)
                pts[kt] = pt

            def pv_step(kt):
                pt = pts.pop(kt)
                nc.tensor.matmul(
                    pv[0][:],
                    vss[b][:, kt, 0:65],
                    pt[:, 0:QT],
                    start=(kt == 0),
                    stop=(kt == NKT - 1),
                )
                nc.tensor.matmul(
                    pv[1][:],
                    vss[b][:, kt, 96:161],
                    pt[:, QT : 2 * QT],
                    start=(kt == 0),
                    stop=(kt == NKT - 1),
                )

            kt0 = 0
            if head is not None:
                # decoupled head: scores+exp for kt0..3 need only K and Q --
                # ACT starts while x_v still streams; the V^T-t0 units and
                # first transposes run before the deferred PV steps
                for kt in range(4):
                    scores_exp(kt)
                for u in head:
                    u()
                for kt in range(4):
                    pv_step(kt)
                kt0 = 4
            for kt in range(kt0, NKT):
                if sched and kt in sched:
                    for u in sched[kt]:
                        u()
                scores_exp(kt)
                if kt < 2 and not (sched and kt in sched):
                    # keep PE fed while PV waits on the previous qt's
                    # normalize chain to free the pv psum buffer
                    pump(1)
                pv_step(kt)
                if not (sched and kt in sched):
                    pump(1 + (len(QuSoft) > 12 and not QuCrit))
            # softmax normalize -> xT (sumexp at psum row 64 for both heads)
            for h in range(HPC):
                se = npool.tile([1, QT], f32, tag="se", name="se")
                nc.vector.tensor_copy(se[:], pv[h][64:65, :])
                rc = npool.tile([1, QT], f32, tag="rc", name="rc")
                nc.vector.reciprocal_approx_fast(rc[:], se[:])
                rb = npool.tile([64, QT], f32, tag="rb", name="rb")
                nc.gpsimd.partition_broadcast(rb[:], rc[:])
                nc.vector.tensor_mul(
                    xTs[b][64 * h : 64 * h + 64, qs], pv[h][0:64, :], rb[:]
                )

        # ================= emission =================
        # -- phase 0: 2MB half loads in first-need order; K/Q tiles 0-1
        # projected directly (both live in the first halves) --
        wo_sb = const.tile([128, D], bf16, tag="wo")
        nc.sync.dma_start(wo_sb[:], wo)
        xdma("k", xk, 0, "h0")()
        xdma("q", xq, 0, "h0")()
        xdma("v", xv, 0, "h0")()
        xdma("k", xk, 0, "h1")()
        xdma("v", xv, 0, "h1")()
        xdma("q", xq, 0, "h1")()
        for u in proj4("k", 0, 0, wk_sb, None, lambda: kTs[0][:, 0:QT]):
            u()
        for u in proj4("k", 0, 1, wk_sb, None, lambda: kTs[0][:, QT : 2 * QT]):
            u()
        for u in proj4("q", 0, 0, wq_sb, bq_sb, lambda: qTs[0][:, 0:QT]):
            u()
        for u in proj4("q", 0, 1, wq_sb, bq_sb, lambda: qTs[0][:, QT : 2 * QT]):
            u()

        # -- b0 qt0: decoupled head (scores/exp kt0-3 first), then a fixed
        # JIT schedule chasing the x DMA stream --
        head0 = list(vt4(0, 0)) + [tp1(0, blk) for blk in range(4)]
        sched0 = {
            4: list(vt4(0, 1)) + [tp1(0, b_) for b_ in range(4, 8)],
            6: proj4("k", 0, 2, wk_sb, None, lambda: kTs[0][:, 2 * QT : 3 * QT]),
            7: vt4(0, 2),
            8: [tp1(0, 8), tp1(0, 9), tp1(0, 10), tp1(0, 11)],
            10: proj4("k", 0, 3, wk_sb, None, lambda: kTs[0][:, 3 * QT : 4 * QT]),
            11: vt4(0, 3),
            12: [tp1(0, 12), tp1(0, 13), tp1(0, 14), tp1(0, 15)],
        }
        attention_qt(0, 0, sched0, head=head0)

        QuSoft.extend(out_unit(0, 0, ot) for ot in range(KC))
        QuCrit.append(xdma("k", xk, 1))
        QuCrit.extend(proj4("q", 0, 2, wq_sb, bq_sb, lambda: qTs[0][:, 2 * QT : 3 * QT]))
        QuCrit.extend(proj4("q", 0, 3, wq_sb, bq_sb, lambda: qTs[0][:, 3 * QT : 4 * QT]))
        attention_qt(0, 1)

        QuSoft.extend(out_unit(0, 1, ot) for ot in range(KC))
        QuCrit.append(xdma("v", xv, 1))
        QuCrit.append(xdma("q", xq, 1))
        QuCrit.extend(proj4("k", 1, 0, wk_sb, None, lambda: kTs[1][:, 0:QT]))
        QuCrit.extend(proj4("k", 1, 1, wk_sb, None, lambda: kTs[1][:, QT : 2 * QT]))
        QuCrit.extend(proj4("k", 1, 2, wk_sb, None, lambda: kTs[1][:, 2 * QT : 3 * QT]))
        QuCrit.extend(proj4("k", 1, 3, wk_sb, None, lambda: kTs[1][:, 3 * QT : 4 * QT]))
        attention_qt(0, 2)

        QuSoft.extend(out_unit(0, 2, ot) for ot in range(KC))
        QuCrit.extend(vt4(1, 0))
        QuCrit.extend(vt4(1, 1))
        QuCrit.extend(vt4(1, 2))
        QuCrit.extend(vt4(1, 3))
        attention_qt(0, 3)

        QuSoft.extend(out_unit(0, 3, ot) for ot in range(KC))
        QuCrit.extend(tp1(1, blk) for blk in range(0, 4))
        QuCrit.extend(proj4("q", 1, 0, wq_sb, bq_sb, lambda: qTs[1][:, 0:QT]))
        drain_crit()
        sched1 = {
            2: [tp1(1, 4), tp1(1, 5), tp1(1, 6), tp1(1, 7)],
            6: [tp1(1, 8), tp1(1, 9), tp1(1, 10), tp1(1, 11)],
            10: [tp1(1, 12), tp1(1, 13), tp1(1, 14), tp1(1, 15)],
        }
        attention_qt(1, 0, sched1)

        QuSoft.extend(out_unit(1, 0, ot) for ot in range(KC))
        QuCrit.extend(proj4("q", 1, 1, wq_sb, bq_sb, lambda: qTs[1][:, QT : 2 * QT]))
        drain_crit()
        attention_qt(1, 1)

        QuSoft.extend(out_unit(1, 1, ot) for ot in range(KC))
        QuCrit.extend(proj4("q", 1, 2, wq_sb, bq_sb, lambda: qTs[1][:, 2 * QT : 3 * QT]))
        drain_crit()
        attention_qt(1, 2)

        QuSoft.extend(out_unit(1, 2, ot) for ot in range(KC))
        QuCrit.extend(proj4("q", 1, 3, wq_sb, bq_sb, lambda: qTs[1][:, 3 * QT : 4 * QT]))
        drain_crit()
        attention_qt(1, 3)

        # tail: ACT is idle after the last exp -- alternate the psum->sbuf
        # copies between Scalar and Vector so the drain pipelines
        QuSoft.extend(out_unit(1, 3, ot, scalar_copy=(ot % 2 == 0)) for ot in range(KC))
        while QuSoft:
            QuSoft.popleft()()

    nc.compile()
    _cache["nc"] = nc
    return nc


def kernel(x_q, x_k, x_v, Wq, bq, Wk, bk, Wv, bv, Wo, bo, _trace=False):
    x_q = np.asarray(x_q, dtype=np.float32)
    x_k = np.asarray(x_k, dtype=np.float32)
    x_v = np.asarray(x_v, dtype=np.float32)
    Wq, Wk, Wv, Wo = (np.asarray(w, dtype=np.float32) for w in (Wq, Wk, Wv, Wo))
    bq, bk, bv, bo = (np.asarray(v, dtype=np.float32) for v in (bq, bk, bv, bo))

    bf = ml_dtypes.bfloat16
    xqT = np.ascontiguousarray(x_q.reshape(R, D).T).astype(bf)
    xkT = np.ascontiguousarray(x_k.reshape(R, D).T).astype(bf)
    xvT = np.ascontiguousarray(x_v.reshape(R, D).T).astype(bf)
    ident = np.eye(128, dtype=bf)

    in_maps = []
    for c in range(NC):
        s = slice(DHC * c, DHC * (c + 1))
        in_maps.append(
            {
                "xqT": xqT,
                "xkT": xkT,
                "xvT": xvT,
                "wqT": _foldw(Wq[s, :].T).astype(bf),
                "wkT": _foldw(Wk[s, :].T).astype(bf),
                "wvT": _foldw(Wv[s, :].T).astype(bf),
                "woT": np.ascontiguousarray(Wo[:, s].T).astype(bf),
                "bq": bq[s][:, None].copy(),
                "ident": ident,
            }
        )

    nc = build()
    res = run_bass_kernel_spmd(nc, in_maps, core_ids=list(range(NC)), trace=_trace)
    total = np.zeros((D, R), dtype=np.float32)
    for c in range(NC):
        total += res.results[c]["outT"].astype(np.float32)
    # bk cancels in softmax; bv's contribution is the constant Wo @ bv
    out = total.T + (bo + Wo @ bv)[None, :]
    if _trace:
        kernel.last_exec_time_ns = res.exec_time_ns
    return out.reshape(B, N, D).astype(np.float32)


# revision 21
# speedup vs baseline: 1.0257x; 1.0257x over previous
"""Multi-head attention (B=2, N=2048, D=1024, H=16) on 8 TRN2 NeuronCores.

Sharding: tensor-parallel over heads. Core c owns heads 2c, 2c+1 (a 128-wide
slice of the concat head dim). Host sums the 8 partial outputs and adds bo.

Structure (v4 — ACT-paced co-schedule):
  - K/Q/V all projected in transposed layout [dh, rows] via 512-col-stream
    matmuls (x moving, weights stationary).  V^T is then turned into PV
    layout [keys, dh] with PE identity-transposes ([128,128] bf16, ~250ns
    each) + two DVE copies per 128-key block — this avoids both the
    ldweights-bound natural V projection and the 1.2us-per-issue XBAR
    transpose DMAs that serialize the sync queue.
  - bk is dropped (softmax-invariant); bv is folded into bo on the host
    (bo' = bo + Wo @ bv); bq is applied in the Q psum->sbuf copy.
  - scores S^T[k,q] for the two heads run CONCURRENTLY on disjoint PE
    row-groups (64-deep contraction, auto tile_position).  exp on ScalarE
    (scale=1/8 folded, no max-subtract), denominator via ones columns in the
    V tile (col 64 for h0, col 160 for h1 -> sumexp at psum row 64 for both).
  - softmax normalize: copy sumexp psum row -> sbuf partition 0, fast
    reciprocal, gpsimd partition_broadcast, one DVE multiply per head
    writing xT (h1 is a cross-partition write, psum parts 0:64 -> 64:128).
  - The exp stream (ACT, ~1.1us per key-tile, 142us total) and the PE
    matmul stream (~145us) are nearly equal; attention key-tiles and
    projection / out-projection units are interleaved at ~0.4us granularity
    so neither engine starves (PE idle triggers HAM half-clock throttling).
    qt0 of each batch runs a fixed JIT schedule chasing the x DMAs (1MB
    half-tile loads, no per-chunk splits: each dma_start costs ~1.1us of
    queue issue time); out-projections are deferred to pad the tail.
"""

import sys

sys.path.insert(0, "/opt/trn_rl_repo")

from collections import deque
from contextlib import ExitStack

import ml_dtypes
import numpy as np

import concourse.bass as bass
import concourse.mybir as mybir
import concourse.tile as tile
from concourse import bacc
from concourse.bass_utils import run_bass_kernel_spmd

B, N, D, H, DH = 2, 2048, 1024, 16, 64
R = B * N  # 4096
NC = 8
HPC = H // NC  # 2 heads per core
DHC = HPC * DH  # 128 head dims per core
QT = 512  # query tile (psum bank)
KT = 128  # key tile (psum partitions)
NQT = N // QT  # 4
NKT = N // KT  # 16
KC = D // 128  # 8 contraction chunks
XW = 2048  # rows per x tile (full batch)

f32 = mybir.dt.float32
bf16 = mybir.dt.bfloat16
EXP = mybir.ActivationFunctionType.Exp

_cache = {}


def _fold(ap):
    # [D, X] dram -> [128, KC, X] partition-folded view
    return ap.rearrange("(a p) m -> p a m", p=128)


def _foldw(w):
    # [D, DHC] host weight -> [128, KC, DHC] partition-folded, contiguous
    return np.ascontiguousarray(w.reshape(KC, 128, DHC).transpose(1, 0, 2))


def build():
    if "nc" in _cache:
        return _cache["nc"]
    nc = bacc.Bacc("TRN2", target_bir_lowering=False, debug=False, num_devices=NC)
    xq = nc.dram_tensor("xqT", [D, R], bf16, kind="ExternalInput").ap()
    xk = nc.dram_tensor("xkT", [D, R], bf16, kind="ExternalInput").ap()
    xv = nc.dram_tensor("xvT", [D, R], bf16, kind="ExternalInput").ap()
    wq = nc.dram_tensor("wqT", [128, KC, DHC], bf16, kind="ExternalInput").ap()
    wk = nc.dram_tensor("wkT", [128, KC, DHC], bf16, kind="ExternalInput").ap()
    wv = nc.dram_tensor("wvT", [128, KC, DHC], bf16, kind="ExternalInput").ap()
    wo = nc.dram_tensor("woT", [DHC, D], bf16, kind="ExternalInput").ap()
    bq = nc.dram_tensor("bq", [DHC, 1], f32, kind="ExternalInput").ap()
    ident = nc.dram_tensor("ident", [128, 128], bf16, kind="ExternalInput").ap()
    outT = nc.dram_tensor("outT", [D, R], bf16, kind="ExternalOutput").ap()

    with tile.TileContext(nc) as tc, ExitStack() as ctx:
        const = ctx.enter_context(tc.tile_pool(name="const", bufs=1))
        xpool = ctx.enter_context(tc.tile_pool(name="x", bufs=3))
        big = ctx.enter_context(tc.tile_pool(name="big", bufs=1))
        vtp = ctx.enter_context(tc.tile_pool(name="vt", bufs=2))
        ppool = ctx.enter_context(tc.tile_pool(name="p", bufs=4))
        obp = ctx.enter_context(tc.tile_pool(name="o", bufs=3))
        npool = ctx.enter_context(tc.tile_pool(name="norm", bufs=2))
        ps_sg = ctx.enter_context(tc.tile_pool(name="psS", bufs=2, space="PSUM"))
        ps_pv = ctx.enter_context(tc.tile_pool(name="psPV", bufs=2, space="PSUM"))
        # proj accumulators, out-proj tiles and V-transposes share this pool
        ps_pj = ctx.enter_context(tc.tile_pool(name="psPJ", bufs=2, space="PSUM"))

        # ---- constants ----
        wk_sb = const.tile([128, KC, DHC], bf16, tag="wk")
        nc.sync.dma_start(wk_sb[:], wk)
        wq_sb = const.tile([128, KC, DHC], bf16, tag="wq")
        nc.sync.dma_start(wq_sb[:], wq)
        wv_sb = const.tile([128, KC, DHC], bf16, tag="wv")
        nc.sync.dma_start(wv_sb[:], wv)
        bq_sb = const.tile([DHC, 1], f32, tag="bq")
        nc.sync.dma_start(bq_sb[:], bq)
        id_sb = const.tile([128, 128], bf16, tag="id")
        nc.sync.dma_start(id_sb[:], ident)

        # ---- persistent activations ----
        qTs, kTs, vss, xTs = [], [], [], []
        for b in range(B):
            qTs.append(big.tile([128, N], bf16, tag=f"qT{b}", name=f"qT{b}"))
            kTs.append(big.tile([128, N], bf16, tag=f"kT{b}", name=f"kT{b}"))
            # per key block: cols 0:64 h0 dims, ones @64, 96:160 h1 dims, ones @160
            v = big.tile([128, NKT, 192], bf16, tag=f"v{b}", name=f"v{b}")
            nc.vector.memset(v[:], 1.0)
            vss.append(v)
            xTs.append(big.tile([128, N], bf16, tag=f"xT{b}", name=f"xT{b}"))

        # ---- x tile management (1MB half-loads / 2MB full loads) ----
        xts = {}

        def xtile(nm, b):
            key = (nm, b)
            if key not in xts:
                xts[key] = xpool.tile([128, KC, XW], bf16, tag="x", name=f"x_{nm}{b}")
            return xts[key]

        # all x loads ride the sync HWDGE in first-need order; aggregate
        # HBM read bandwidth (~350GB/s) is shared across queues, so
        # splitting across queues only fair-shares it away from the
        # critical first tiles (measured: strictly worse)
        def xdma(nm, xdram, b, kind="full"):
            def u():
                xt = xtile(nm, b)
                eng = nc.sync
                if kind == "h0":
                    eng.dma_start(
                        xt[:, :, 0:1024], _fold(xdram[:, b * N : b * N + 1024])
                    )
                elif kind == "h1":
                    eng.dma_start(
                        xt[:, :, 1024:2048],
                        _fold(xdram[:, b * N + 1024 : b * N + 2048]),
                    )
                elif kind == "a3":
                    eng.dma_start(
                        xt[:, :, 0:1536], _fold(xdram[:, b * N : b * N + 1536])
                    )
                elif kind == "b1":
                    eng.dma_start(
                        xt[:, :, 1536:2048],
                        _fold(xdram[:, b * N + 1536 : b * N + 2048]),
                    )
                else:
                    eng.dma_start(xt[:], _fold(xdram[:, b * N : b * N + XW]))

            return u

        # ---- projection quarter-units (2 contraction chunks each) ----
        def proj4(nm, b, t, w_sb, bias, dst_fn):
            st = {}

            def mk(i):
                def u():
                    x = xtile(nm, b)
                    col0 = t * QT
                    if i == 0:
                        st["ps"] = ps_pj.tile([128, QT], f32, tag="pj", name="pspj")
                    ps = st["ps"]
                    for kc in (2 * i, 2 * i + 1):
                        nc.tensor.matmul(
                            ps[:],
                            w_sb[:, kc, :],
                            x[:, kc, col0 : col0 + QT],
                            start=(kc == 0),
                            stop=(kc == KC - 1),
                        )
                    if i == 3:
                        dst = dst_fn()
                        if bias is None:
                            nc.vector.tensor_copy(dst, ps[:])
                        else:
                            nc.vector.tensor_scalar_add(dst, ps[:], bias[:])

                return u

            return [mk(i) for i in range(4)]

        vts = {}

        def vtile(b):
            if b not in vts:
                vts[b] = vtp.tile([128, N], bf16, tag="vt", name=f"vT{b}")
            return vts[b]

        def vt4(b, t):
            return proj4(
                "v", b, t, wv_sb, None, lambda: vtile(b)[:, t * QT : (t + 1) * QT]
            )

        def tp1(b, blk):
            # PE transpose V^T block [128 dhc, 128 keys] -> [keys, dhc] psum,
            # then split the two heads' halves into the vss PV layout
            def u():
                tp = ps_pj.tile([128, 128], bf16, tag="pj", name="pstp")
                nc.tensor.transpose(
                    tp[:], vtile(b)[:, blk * KT : (blk + 1) * KT], id_sb[:]
                )
                nc.vector.tensor_copy(vss[b][:, blk, 0:64], tp[:, 0:64])
                nc.vector.tensor_copy(vss[b][:, blk, 96:160], tp[:, 64:128])

            return u

        def out_unit(b, qt, ot, scalar_copy=False):
            def u():
                ps = ps_pj.tile([128, QT], f32, tag="pj", name="pso")
                nc.tensor.matmul(
                    ps[:],
                    wo_sb[:, ot * 128 : (ot + 1) * 128],
                    xTs[b][:, qt * QT : (qt + 1) * QT],
                    start=True,
                    stop=True,
                )
                ob = obp.tile([128, QT], bf16, tag="ob", name="ob")
                if scalar_copy:
                    nc.scalar.copy(ob[:], ps[:])
                else:
                    nc.vector.tensor_copy(ob[:], ps[:])
                nc.gpsimd.dma_start(
                    outT[
                        ot * 128 : (ot + 1) * 128,
                        b * N + qt * QT : b * N + (qt + 1) * QT,
                    ],
                    ob[:],
                )

            return u

        QuCrit = deque()
        QuSoft = deque()

        def pump(n):
            for _ in range(n):
                if QuCrit:
                    QuCrit.popleft()()
                elif QuSoft:
                    QuSoft.popleft()()

        def drain_crit():
            while QuCrit:
                QuCrit.popleft()()

        # ---- attention for one query tile ----
        def attention_qt(b, qt, sched=None, head=None):
            qs = slice(qt * QT, (qt + 1) * QT)
            pv = [
                ps_pv.tile([65, QT], f32, tag="pv", name=f"pv{h}") for h in range(HPC)
            ]
            pts = {}

            def scores_exp(kt):
                ks = slice(kt * KT, (kt + 1) * KT)
                sg = ps_sg.tile([128, 2 * QT], f32, tag="sg", name="sg")
                for h in range(HPC):
                    hp = slice(64 * h, 64 * h + 64)
                    nc.tensor.matmul(
                        sg[:, h * QT : (h + 1) * QT],
                        kTs[b][hp, ks],
                        qTs[b][hp, qs],
                        start=True,
                        stop=True,
                    )
                pt = ppool.tile([128, 2 * QT], bf16, tag="pt", name="pt")
                nc.scalar.activation(pt[:], sg[:], EXP, scale=0.125)
                pts[kt] = pt

            def pv_step(kt):
                pt = pts.pop(kt)
                nc.tensor.matmul(
                    pv[0][:],
                    vss[b][:, kt, 0:65],
                    pt[:, 0:QT],
                    start=(kt == 0),
                    stop=(kt == NKT - 1),
                )
                nc.tensor.matmul(
                    pv[1][:],
                    vss[b][:, kt, 96:161],
                    pt[:, QT : 2 * QT],
                    start=(kt == 0),
                    stop=(kt == NKT - 1),
                )

            kt0 = 0
            if head is not None:
                # decoupled head: scores+exp for kt0..3 need only K and Q --
                # ACT starts while x_v still streams; the V^T-t0 units and
                # first transposes run before the deferred PV steps
                for kt in range(4):
                    scores_exp(kt)
                for u in head:
                    u()
                for kt in range(4):
                    pv_step(kt)
                kt0 = 4
            for kt in range(kt0, NKT):
                if sched and kt in sched:
                    for u in sched[kt]:
                        u()
                scores_exp(kt)
                if kt < 2 and not (sched and kt in sched):
                    # keep PE fed while PV waits on the previous qt's
                    # normalize chain to free the pv psum buffer
                    pump(1)
                pv_step(kt)
                if not (sched and kt in sched):
                    pump(1 + (len(QuSoft) > 12 and not QuCrit))
            # softmax normalize -> xT (sumexp at psum row 64 for both heads)
            for h in range(HPC):
                se = npool.tile([1, QT], f32, tag="se", name="se")
                nc.vector.tensor_copy(se[:], pv[h][64:65, :])
                rc = npool.tile([1, QT], f32, tag="rc", name="rc")
                nc.vector.reciprocal_approx_fast(rc[:], se[:])
                rb = npool.tile([64, QT], f32, tag="rb", name="rb")
                nc.gpsimd.partition_broadcast(rb[:], rc[:])
                nc.vector.tensor_mul(
                    xTs[b][64 * h : 64 * h + 64, qs], pv[h][0:64, :], rb[:]
                )

        # ================= emission =================
        # -- phase 0: 2MB half loads in first-need order; K/Q tiles 0-1
        # projected directly (both live in the first halves) --
        wo_sb = const.tile([128, D], bf16, tag="wo")
        nc.sync.dma_start(wo_sb[:], wo)
        xdma("k", xk, 0, "h0")()
        xdma("q", xq, 0, "h0")()
        xdma("v", xv, 0, "h0")()
        xdma("k", xk, 0, "h1")()
        xdma("v", xv, 0, "h1")()
        xdma("q", xq, 0, "h1")()
        for u in proj4("k", 0, 0, wk_sb, None, lambda: kTs[0][:, 0:QT]):
            u()
        for u in proj4("k", 0, 1, wk_sb, None, lambda: kTs[0][:, QT : 2 * QT]):
            u()
        for u in proj4("q", 0, 0, wq_sb, bq_sb, lambda: qTs[0][:, 0:QT]):
            u()
        for u in proj4("q", 0, 1, wq_sb, bq_sb, lambda: qTs[0][:, QT : 2 * QT]):
            u()

        # -- b0 qt0: decoupled head (scores/exp kt0-3 first), then a fixed
        # JIT schedule chasing the x DMA stream --
        head0 = list(vt4(0, 0)) + [tp1(0, blk) for blk in range(4)]
        sched0 = {
            4: list(vt4(0, 1)) + [tp1(0, b_) for b_ in range(4, 8)],
            6: proj4("k", 0, 2, wk_sb, None, lambda: kTs[0][:, 2 * QT : 3 * QT]),
            7: vt4(0, 2),
            8: [tp1(0, 8), tp1(0, 9), tp1(0, 10), tp1(0, 11)],
            10: proj4("k", 0, 3, wk_sb, None, lambda: kTs[0][:, 3 * QT : 4 * QT]),
            11: vt4(0, 3),
            12: [tp1(0, 12), tp1(0, 13), tp1(0, 14), tp1(0, 15)],
        }
        attention_qt(0, 0, sched0, head=head0)

        QuSoft.extend(out_unit(0, 0, ot) for ot in range(KC))
        QuCrit.append(xdma("k", xk, 1))
        QuCrit.extend(proj4("q", 0, 2, wq_sb, bq_sb, lambda: qTs[0][:, 2 * QT : 3 * QT]))
        QuCrit.extend(proj4("q", 0, 3, wq_sb, bq_sb, lambda: qTs[0][:, 3 * QT : 4 * QT]))
        attention_qt(0, 1)

        QuSoft.extend(out_unit(0, 1, ot) for ot in range(KC))
        QuCrit.append(xdma("v", xv, 1))
        QuCrit.append(xdma("q", xq, 1))
        QuCrit.extend(proj4("k", 1, 0, wk_sb, None, lambda: kTs[1][:, 0:QT]))
        QuCrit.extend(proj4("k", 1, 1, wk_sb, None, lambda: kTs[1][:, QT : 2 * QT]))
        QuCrit.extend(proj4("k", 1, 2, wk_sb, None, lambda: kTs[1][:, 2 * QT : 3 * QT]))
        QuCrit.extend(proj4("k", 1, 3, wk_sb, None, lambda: kTs[1][:, 3 * QT : 4 * QT]))
        attention_qt(0, 2)

        QuSoft.extend(out_unit(0, 2, ot) for ot in range(KC))
        QuCrit.extend(vt4(1, 0))
        QuCrit.extend(vt4(1, 1))
        QuCrit.extend(vt4(1, 2))
        QuCrit.extend(vt4(1, 3))
        attention_qt(0, 3)

        QuSoft.extend(out_unit(0, 3, ot) for ot in range(KC))
        QuCrit.extend(tp1(1, blk) for blk in range(0, 4))
        QuCrit.extend(proj4("q", 1, 0, wq_sb, bq_sb, lambda: qTs[1][:, 0:QT]))
        drain_crit()
        sched1 = {
            2: [tp1(1, 4), tp1(1, 5), tp1(1, 6), tp1(1, 7)],
            6: [tp1(1, 8), tp1(1, 9), tp1(1, 10), tp1(1, 11)],
            10: [tp1(1, 12), tp1(1, 13), tp1(1, 14), tp1(1, 15)],
        }
        attention_qt(1, 0, sched1)

        QuSoft.extend(out_unit(1, 0, ot) for ot in range(KC))
        QuCrit.extend(proj4("q", 1, 1, wq_sb, bq_sb, lambda: qTs[1][:, QT : 2 * QT]))
        drain_crit()
        attention_qt(1, 1)

        QuSoft.extend(out_unit(1, 1, ot) for ot in range(KC))
        QuCrit.extend(proj4("q", 1, 2, wq_sb, bq_sb, lambda: qTs[1][:, 2 * QT : 3 * QT]))
        drain_crit()
        attention_qt(1, 2)

        QuSoft.extend(out_unit(1, 2, ot) for ot in range(KC))
        QuCrit.extend(proj4("q", 1, 3, wq_sb, bq_sb, lambda: qTs[1][:, 3 * QT : 4 * QT]))
        drain_crit()
        attention_qt(1, 3)

        # tail: ACT is idle after the last exp -- alternate the psum->sbuf
        # copies between Scalar and Vector so the drain pipelines
        QuSoft.extend(out_unit(1, 3, ot, scalar_copy=(ot % 2 == 0)) for ot in range(KC))
        while QuSoft:
            QuSoft.popleft()()

    nc.compile()
    _cache["nc"] = nc
    return nc


def kernel(x_q, x_k, x_v, Wq, bq, Wk, bk, Wv, bv, Wo, bo, _trace=False):
    x_q = np.asarray(x_q, dtype=np.float32)
    x_k = np.asarray(x_k, dtype=np.float32)
    x_v = np.asarray(x_v, dtype=np.float32)
    Wq, Wk, Wv, Wo = (np.asarray(w, dtype=np.float32) for w in (Wq, Wk, Wv, Wo))
    bq, bk, bv, bo = (np.asarray(v, dtype=np.float32) for v in (bq, bk, bv, bo))

    bf = ml_dtypes.bfloat16
    xqT = np.ascontiguousarray(x_q.reshape(R, D).T).astype(bf)
    xkT = np.ascontiguousarray(x_k.reshape(R, D).T).astype(bf)
    xvT = np.ascontiguousarray(x_v.reshape(R, D).T).astype(bf)
    ident = np.eye(128, dtype=bf)

    in_maps = []
    for c in range(NC):
        s = slice(DHC * c, DHC * (c + 1))
        in_maps.append(
            {
                "xqT": xqT,
                "xkT": xkT,
                "xvT": xvT,
                "wqT": _foldw(Wq[s, :].T).astype(bf),
                "wkT": _foldw(Wk[s, :].T).astype(bf),
                "wvT": _foldw(Wv[s, :].T).astype(bf),
                "woT": np.ascontiguousarray(Wo[:, s].T).astype(bf),
                "bq": bq[s][:, None].copy(),
                "ident": ident,
            }
        )

    nc = build()
    res = run_bass_kernel_spmd(nc, in_maps, core_ids=list(range(NC)), trace=_trace)
    total = np.zeros((D, R), dtype=np.float32)
    for c in range(NC):
        total += res.results[c]["outT"].astype(np.float32)
    # bk cancels in softmax; bv's contribution is the constant Wo @ bv
    out = total.T + (bo + Wo @ bv)[None, :]
    if _trace:
        kernel.last_exec_time_ns = res.exec_time_ns
    return out.reshape(B, N, D).astype(np.float32)
